# revision 27
# baseline (speedup 1.0000x reference)
"""Trainium2 Bass kernel for the controlled-unitary problem.

reference semantics (control=0, num_qubits=13, dim=8192):
    mask bit = 1 << 12, so columns/rows with that bit set are idx 4096..8191.
    out[:, c0] = state[:, c0]                       (control bit off: untouched)
    out[:, c1] = state[:, c1] @ target[c1, c1]      (controlled unitary)

Device work: complex [256,4096] @ [4096,4096] GEMM.
Sharding: output columns of the GEMM split 8 ways (each core gets a
[4096, 512] slab of the target block; every weight byte moves once).

Per-core kernel (v5):
  - Karatsuba split: t1 = ar.br, t2 = ai.bi, t3 = (ar+ai).(br+bi);
    C_r = t1 - t2, C_i = t3 - t1 - t2. One derived plane per operand.
  - All inputs fully resident in SBUF; every DMA issued upfront.
  - HWDGE rings carry only chunk0 of A plus the B planes (SP ring:
    a_r c0 + b_r chunks; ACT ring: a_i c0 + b_i chunks) so early
    issue serialization never delays a B chunk.  A k1..31 rides the
    SWDGE (gpsimd) queue as 3 coalesced transfers per plane (one
    contiguous descriptor per partition).
  - Warmup matmuls on garbage SBUF (no memset dependency) ramp the
    PE p-state from body entry while input DMA streams.
  - Last chunk is m-major; w = t1s + t2 is precomputed so only ONE
    DVE op (out_i = t3 - w) remains after the final matmul; m1's t3
    is N-split so the first half stores while the second computes.
  - fp16 outputs halve the store traffic.
"""

import os

import numpy as np

BATCH = 256
DIM = 8192
HALF = 4096
N_CORES = 8
NSH = HALF // N_CORES  # 512 output columns per core
KT = HALF // 128  # 32 k-tiles
MT = BATCH // 128  # 2 m-tiles

DT_NAME = os.environ.get("KERNEL_DT", "float16")
CHUNKS = [int(x) for x in os.environ.get(
    "KERNEL_CHUNKS", "1,1,2,2,3,4,5,6,8").split(",")]
assert sum(CHUNKS) == KT
NWARM = int(os.environ.get("KERNEL_NWARM", "6"))
OUT_DT_NAME = os.environ.get("KERNEL_OUT_DT", "float16")
# A-plane segment boundaries (coalesced HWDGE issues; fewer, bigger
# issues than the per-chunk B stream so B chunks are never stuck
# behind a queue of A issues)
A_SEG = [int(x) for x in os.environ.get(
    "KERNEL_A_SEG", "0,1,2,6,14,22,32").split(",")]
# after how many B-chunk issues each A segment issue is placed
A_POS = [int(x) for x in os.environ.get(
    "KERNEL_A_POS", "0,1,2,4,6,8").split(",")]
assert len(A_POS) == len(A_SEG) - 1
# 0 = no tail split, N>=2 = split m1-t3 inside the last chunk into N
# sequential column groups, each with its own epilogue + store
NSPLIT = int(os.environ.get("KERNEL_NSPLIT", "2"))
# B chunks with index < BHALF_CHUNKS are transferred and computed as
# two N-halves (measured: regression — issue pressure outweighs the
# earlier first-half arrival; keep 0)
BHALF_CHUNKS = int(os.environ.get("KERNEL_BHALF_CHUNKS", "0"))

_CACHE = {}


def _np_dtype(dt_name):
    return np.float16 if dt_name == "float16" else np.float32


def _build(dt_name):
    import concourse.mybir as mybir
    import concourse.tile as tile
    from concourse import bacc

    DT = getattr(mybir.dt, dt_name)
    ODT = getattr(mybir.dt, OUT_DT_NAME)
    F32 = mybir.dt.float32

    nc = bacc.Bacc("TRN2", target_bir_lowering=False, debug=False,
                   num_devices=N_CORES)

    a_r = nc.dram_tensor("a_r", [128, KT, BATCH], DT, kind="ExternalInput")
    a_i = nc.dram_tensor("a_i", [128, KT, BATCH], DT, kind="ExternalInput")
    b_r = nc.dram_tensor("b_r", [128, KT, NSH], DT, kind="ExternalInput")
    b_i = nc.dram_tensor("b_i", [128, KT, NSH], DT, kind="ExternalInput")
    c_r = nc.dram_tensor("c_r", [BATCH, NSH], ODT, kind="ExternalOutput")
    c_i = nc.dram_tensor("c_i", [BATCH, NSH], ODT, kind="ExternalOutput")

    with tile.TileContext(nc) as tc:
        with (
            tc.tile_pool(name="sb", bufs=1) as sb,
            tc.tile_pool(name="ps", bufs=1, space="PSUM") as ps_pool,
        ):
            A_r = sb.tile([128, KT, BATCH], DT, name="A_r")
            A_i = sb.tile([128, KT, BATCH], DT, name="A_i")
            A_s = sb.tile([128, KT, BATCH], DT, name="A_s")
            B_r = sb.tile([128, KT, NSH], DT, name="B_r")
            B_i = sb.tile([128, KT, NSH], DT, name="B_i")
            B_s = sb.tile([128, KT, NSH], DT, name="B_s")
            warm_w = sb.tile([128, 128], DT, name="warm_w")

            ps = {}
            for m in range(MT):
                for t in ("t1", "t2", "t3"):
                    if NSPLIT != 0 and m == MT - 1 and t == "t3":
                        continue
                    ps[(m, t)] = ps_pool.tile([128, NSH], F32,
                                              name=f"ps_{m}_{t}")
            if NSPLIT != 0:
                # m1-t3 accumulates as two N-halves in separate banks
                # so the PE never writes a bank the DVE is reading
                # during the tail
                HN = NSH // 2
                ps_t3h = [
                    ps_pool.tile([128, NSH], F32, name=f"ps_t3h{h}")
                    for h in range(2)
                ]
            ps_warm = ps_pool.tile([128, NSH], F32, name="ps_warm")

            # PE warmup: one small scratch memset on the DVE queue
            # (idle until the first A_s prep ~2us later); N=128 warmup
            # matmuls ramp the HAM clock-gate while input DMA streams,
            # with fine granularity at the warmup -> real handoff.
            nc.vector.memset(warm_w[:], 0.0)
            for _ in range(NWARM):
                nc.tensor.matmul(ps_warm[:, 0:128], warm_w[:], warm_w[:],
                                 start=True, stop=True)

            # input DMA issue sequence: per-chunk B issues with the
            # coalesced A segment issues interleaved at A_POS
            k0 = 0
            b_slices = []
            for ch in CHUNKS:
                b_slices.append(slice(k0, k0 + ch))
                k0 += ch
            a_slices = [slice(lo, hi) for lo, hi in zip(A_SEG[:-1], A_SEG[1:])]
            n_chunks = len(CHUNKS)
            BH = min(BHALF_CHUNKS, n_chunks - 1)
            HN = NSH // 2
            issue_order = []  # ("A"|"B"|"B0"|"B1", slice)
            ai = 0
            for bi_, bsl in enumerate(b_slices):
                while ai < len(a_slices) and A_POS[ai] <= bi_:
                    issue_order.append(("A", a_slices[ai]))
                    ai += 1
                if bi_ < BH:
                    issue_order.append(("B0", bsl))
                    if bi_ == BH - 1:
                        for bsl2 in b_slices[:BH]:
                            issue_order.append(("B1", bsl2))
                else:
                    issue_order.append(("B", bsl))
            while ai < len(a_slices):
                issue_order.append(("A", a_slices[ai]))
                ai += 1
            for kind, sl in issue_order:
                if kind == "A":
                    nc.sync.dma_start(A_r[:, sl], a_r[:, sl])
                    nc.scalar.dma_start(A_i[:, sl], a_i[:, sl])
                elif kind == "B":
                    nc.sync.dma_start(B_r[:, sl], b_r[:, sl])
                    nc.scalar.dma_start(B_i[:, sl], b_i[:, sl])
                else:
                    h = 0 if kind == "B0" else 1
                    nsl = slice(h * HN, (h + 1) * HN)
                    nc.sync.dma_start(B_r[:, sl, nsl], b_r[:, sl, nsl])
                    nc.scalar.dma_start(B_i[:, sl, nsl], b_i[:, sl, nsl])

            operands = {
                "t1": (A_r, B_r),
                "t2": (A_i, B_i),
                "t3": (A_s, B_s),
            }

            # early chunks in two N-half phases: the h0 matmuls of each
            # k-tile need only half the B bytes, so the PE rides closer
            # behind the DMA completion line
            chunk_k0 = [s.start for s in b_slices]
            for h in range(2):
                nsl = slice(h * HN, (h + 1) * HN)
                for ci in range(BH):
                    ksl = b_slices[ci]
                    k0 = chunk_k0[ci]
                    ch = ksl.stop - ksl.start
                    if h == 0:
                        nc.vector.tensor_tensor(
                            A_s[:, ksl], A_r[:, ksl], A_i[:, ksl],
                            mybir.AluOpType.add)
                    nc.vector.tensor_tensor(
                        B_s[:, ksl, nsl], B_r[:, ksl, nsl],
                        B_i[:, ksl, nsl], mybir.AluOpType.add)
                    for t in ("t1", "t2", "t3"):
                        lhs, rhs = operands[t]
                        for kk in range(ch):
                            k = k0 + kk
                            for m in range(MT):
                                msl = slice(m * 128, (m + 1) * 128)
                                if NSPLIT != 0 and m == MT - 1 and t == "t3":
                                    nc.tensor.matmul(
                                        ps_t3h[h][:, 0:HN],
                                        lhs[:, k, msl], rhs[:, k, nsl],
                                        start=(k == 0), stop=False,
                                    )
                                else:
                                    # h1 into an h0-started bank must
                                    # NOT use start (it would clear the
                                    # whole bank's has_written bits);
                                    # untouched columns overwrite anyway
                                    nc.tensor.matmul(
                                        ps[(m, t)][:, nsl],
                                        lhs[:, k, msl], rhs[:, k, nsl],
                                        start=(h == 0 and k == 0),
                                        stop=False,
                                        skip_group_check=True,
                                    )

            k0 = chunk_k0[BH] if BH < n_chunks else KT
            for ci in range(BH, n_chunks):
                ch = CHUNKS[ci]
                ksl = slice(k0, k0 + ch)
                nc.vector.tensor_tensor(A_s[:, ksl], A_r[:, ksl],
                                        A_i[:, ksl], mybir.AluOpType.add)
                nc.vector.tensor_tensor(B_s[:, ksl], B_r[:, ksl],
                                        B_i[:, ksl], mybir.AluOpType.add)

                last_chunk = ci == n_chunks - 1
                if not last_chunk:
                    # product-major: t1 needs only A_r + B_r, so the PE
                    # starts before A_i/B_i/prep land
                    for t in ("t1", "t2", "t3"):
                        lhs, rhs = operands[t]
                        for kk in range(ch):
                            k = k0 + kk
                            for m in range(MT):
                                msl = slice(m * 128, (m + 1) * 128)
                                if NSPLIT != 0 and m == MT - 1 and t == "t3":
                                    for h in range(2):
                                        nsl = slice(h * HN, (h + 1) * HN)
                                        nc.tensor.matmul(
                                            ps_t3h[h][:, 0:HN],
                                            lhs[:, k, msl],
                                            rhs[:, k, nsl],
                                            start=(k == 0), stop=False,
                                        )
                                else:
                                    nc.tensor.matmul(
                                        ps[(m, t)][:], lhs[:, k, msl],
                                        rhs[:, k, :], start=(k == 0),
                                        stop=False,
                                        skip_group_check=True,
                                    )
                else:
                    # m-major: finish all of m0 first so its epilogue
                    # overlaps m1's matmuls; per product, issue the
                    # epilogue ops that only need completed banks.
                    for m in range(MT):
                        msl = slice(m * 128, (m + 1) * 128)
                        t1s = sb.tile([128, NSH], F32, name=f"t1s{m}")
                        w = sb.tile([128, NSH], F32, name=f"w{m}")
                        out_r = sb.tile([128, NSH], ODT, name=f"out_r{m}")
                        out_i = sb.tile([128, NSH], ODT, name=f"out_i{m}")
                        split = NSPLIT != 0 and m == MT - 1
                        for t in ("t1", "t2", "t3"):
                            lhs, rhs = operands[t]
                            if t == "t3" and split:
                                HN = NSH // 2
                                for h in range(2):
                                    nsl = slice(h * HN, (h + 1) * HN)
                                    dst = ps_t3h[h][:, 0:HN]
                                    for kk in range(ch):
                                        k = k0 + kk
                                        nc.tensor.matmul(
                                            dst,
                                            lhs[:, k, msl],
                                            rhs[:, k, nsl],
                                            start=(k == 0),
                                            stop=(kk == ch - 1),
                                        )
                                    nc.vector.tensor_tensor(
                                        out_i[:, nsl], dst,
                                        w[:, nsl],
                                        mybir.AluOpType.subtract)
                                    # route the two half-stores via
                                    # different queues so the last
                                    # issue isn't stuck behind the
                                    # first on one engine
                                    eng = nc.scalar if h == 0 else nc.sync
                                    eng.dma_start(
                                        c_i[msl, nsl], out_i[:, nsl])
                                continue
                            for kk in range(ch):
                                k = k0 + kk
                                nc.tensor.matmul(
                                    ps[(m, t)][:], lhs[:, k, msl],
                                    rhs[:, k, :], start=(k == 0),
                                    stop=(kk == ch - 1),
                                    skip_group_check=True,
                                )
                            if t == "t1":
                                nc.scalar.copy(t1s[:], ps[(m, "t1")][:])
                            elif t == "t2":
                                # C_r = t1 - t2; w = t1 + t2
                                nc.vector.tensor_tensor(
                                    out_r[:], t1s[:], ps[(m, "t2")][:],
                                    mybir.AluOpType.subtract)
                                nc.sync.dma_start(c_r[msl, :], out_r[:])
                                nc.vector.tensor_tensor(
                                    w[:], t1s[:], ps[(m, "t2")][:],
                                    mybir.AluOpType.add)
                            else:
                                # C_i = t3 - t1 - t2 = t3 - w
                                nc.vector.tensor_tensor(
                                    out_i[:], ps[(m, "t3")][:], w[:],
                                    mybir.AluOpType.subtract)
                                nc.scalar.dma_start(
                                    c_i[msl, :], out_i[:])
                k0 += ch

    nc.compile()
    return nc


def _get_nc(dt_name):
    if dt_name not in _CACHE:
        _CACHE[dt_name] = _build(dt_name)
    return _CACHE[dt_name]


def _pack_kxm(mat_t, np_dt):
    # mat_t: [4096, F] (k-major) -> [128, KT, F] with k = kt*128 + p
    f = mat_t.shape[1]
    return np.ascontiguousarray(
        mat_t.reshape(KT, 128, f).transpose(1, 0, 2).astype(np_dt)
    )


def run_device(A, B, dt_name=DT_NAME, trace=False):
    """A: [256, 4096] complex64, B: [4096, 4096] complex64.
    Returns C = A @ B as [256, 4096] complex64 plus the raw results."""
    from concourse import bass_utils

    nc = _get_nc(dt_name)
    np_dt = _np_dtype(dt_name)

    at = A.T  # [4096, 256]
    a_r = _pack_kxm(np.ascontiguousarray(at.real), np_dt)
    a_i = _pack_kxm(np.ascontiguousarray(at.imag), np_dt)
    br_full = B.real
    bi_full = B.imag

    in_maps = []
    for c in range(N_CORES):
        csl = slice(c * NSH, (c + 1) * NSH)
        in_maps.append({
            "a_r": a_r,
            "a_i": a_i,
            "b_r": _pack_kxm(np.ascontiguousarray(br_full[:, csl]), np_dt),
            "b_i": _pack_kxm(np.ascontiguousarray(bi_full[:, csl]), np_dt),
        })

    res = bass_utils.run_bass_kernel_spmd(
        nc, in_maps, core_ids=list(range(N_CORES)), trace=trace
    )

    out = np.empty((BATCH, HALF), dtype=np.complex64)
    for c in range(N_CORES):
        csl = slice(c * NSH, (c + 1) * NSH)
        out.real[:, csl] = res.results[c]["c_r"].astype(np.float32)
        out.imag[:, csl] = res.results[c]["c_i"].astype(np.float32)
    return out, res


def kernel(state, target_matrix, control, num_qubits):
    state = np.asarray(state)
    target_matrix = np.asarray(target_matrix)
    control = int(control)
    num_qubits = int(num_qubits)
    dim = 1 << num_qubits

    assert state.shape == (BATCH, DIM) and dim == DIM, (
        "kernel hardcoded for [256, 8192]"
    )

    mask = 1 << (num_qubits - control - 1)
    idx = np.arange(dim)
    c1 = idx[(idx & mask) != 0]  # columns with control bit set

    if control == 0:
        A = state[:, HALF:]
        B = target_matrix[HALF:, HALF:]
    else:
        A = state[:, c1]
        B = target_matrix[np.ix_(c1, c1)]
    A = np.ascontiguousarray(A, dtype=np.complex64)
    B = np.ascontiguousarray(B, dtype=np.complex64)

    C, _ = run_device(A, B)

    out = state.astype(np.complex64, copy=True)
    out[:, c1] = C
    return out


# revision 29
# speedup vs baseline: 1.0169x; 1.0169x over previous
"""Trainium2 Bass kernel for the controlled-unitary problem.

reference semantics (control=0, num_qubits=13, dim=8192):
    mask bit = 1 << 12, so columns/rows with that bit set are idx 4096..8191.
    out[:, c0] = state[:, c0]                       (control bit off: untouched)
    out[:, c1] = state[:, c1] @ target[c1, c1]      (controlled unitary)

Device work: complex [256,4096] @ [4096,4096] GEMM.
Sharding: output columns of the GEMM split 8 ways (each core gets a
[4096, 512] slab of the target block; every weight byte moves once).

Per-core kernel (v6):
  - Karatsuba split: t1 = ar.br, t2 = ai.bi, t3 = (ar+ai).(br+bi);
    C_r = t1 - t2, C_i = t3 - t1 - t2. One derived plane per operand.
  - All inputs fully resident in SBUF; every DMA issued upfront on
    the two HWDGE rings (SP: a_r/b_r, ACT: a_i/b_i).  B moves as
    per-chunk issues in k-order; A moves as ~6 coalesced segment
    issues interleaved at matching k positions, so early B chunks
    are never stuck behind a queue of A issues.
  - N=128 warmup matmuls on a memset scratch tile ramp the PE HAM
    clock-gate from body entry while input DMA streams; NWARM sized
    so the real matmuls start just behind the DMA-visibility line.
  - m1's t3 accumulates as two N-halves in separate PSUM banks; in
    the m-major last chunk each half finishes, combines (one DVE op:
    out_i = t3 - w) and stores independently on different queues,
    shrinking the post-matmul tail.
  - fp16 outputs halve the store traffic.
"""

import os

import numpy as np

BATCH = 256
DIM = 8192
HALF = 4096
N_CORES = 8
NSH = HALF // N_CORES  # 512 output columns per core
KT = HALF // 128  # 32 k-tiles
MT = BATCH // 128  # 2 m-tiles

DT_NAME = os.environ.get("KERNEL_DT", "float16")
CHUNKS = [int(x) for x in os.environ.get(
    "KERNEL_CHUNKS", "1,1,2,2,3,4,5,6,8").split(",")]
assert sum(CHUNKS) == KT
NWARM = int(os.environ.get("KERNEL_NWARM", "50"))
OUT_DT_NAME = os.environ.get("KERNEL_OUT_DT", "float16")
# A-plane segment boundaries (coalesced HWDGE issues; fewer, bigger
# issues than the per-chunk B stream so B chunks are never stuck
# behind a queue of A issues)
A_SEG = [int(x) for x in os.environ.get(
    "KERNEL_A_SEG", "0,1,2,6,14,22,32").split(",")]
# after how many B-chunk issues each A segment issue is placed
A_POS = [int(x) for x in os.environ.get(
    "KERNEL_A_POS", "0,1,2,4,6,8").split(",")]
assert len(A_POS) == len(A_SEG) - 1
# 0 = no tail split, N>=2 = split m1-t3 inside the last chunk into N
# sequential column groups, each with its own epilogue + store
NSPLIT = int(os.environ.get("KERNEL_NSPLIT", "2"))
# B chunks with index < BHALF_CHUNKS are transferred and computed as
# two N-halves (measured: regression — issue pressure outweighs the
# earlier first-half arrival; keep 0)
BHALF_CHUNKS = int(os.environ.get("KERNEL_BHALF_CHUNKS", "0"))

_CACHE = {}


def _np_dtype(dt_name):
    return np.float16 if dt_name == "float16" else np.float32


def _build(dt_name):
    import concourse.mybir as mybir
    import concourse.tile as tile
    from concourse import bacc

    DT = getattr(mybir.dt, dt_name)
    ODT = getattr(mybir.dt, OUT_DT_NAME)
    F32 = mybir.dt.float32

    nc = bacc.Bacc("TRN2", target_bir_lowering=False, debug=False,
                   num_devices=N_CORES)

    a_r = nc.dram_tensor("a_r", [128, KT, BATCH], DT, kind="ExternalInput")
    a_i = nc.dram_tensor("a_i", [128, KT, BATCH], DT, kind="ExternalInput")
    b_r = nc.dram_tensor("b_r", [128, KT, NSH], DT, kind="ExternalInput")
    b_i = nc.dram_tensor("b_i", [128, KT, NSH], DT, kind="ExternalInput")
    c_r = nc.dram_tensor("c_r", [BATCH, NSH], ODT, kind="ExternalOutput")
    c_i = nc.dram_tensor("c_i", [BATCH, NSH], ODT, kind="ExternalOutput")

    with tile.TileContext(nc) as tc:
        with (
            tc.tile_pool(name="sb", bufs=1) as sb,
            tc.tile_pool(name="ps", bufs=1, space="PSUM") as ps_pool,
        ):
            A_r = sb.tile([128, KT, BATCH], DT, name="A_r")
            A_i = sb.tile([128, KT, BATCH], DT, name="A_i")
            A_s = sb.tile([128, KT, BATCH], DT, name="A_s")
            B_r = sb.tile([128, KT, NSH], DT, name="B_r")
            B_i = sb.tile([128, KT, NSH], DT, name="B_i")
            B_s = sb.tile([128, KT, NSH], DT, name="B_s")
            warm_w = sb.tile([128, 128], DT, name="warm_w")

            ps = {}
            for m in range(MT):
                for t in ("t1", "t2", "t3"):
                    if NSPLIT != 0 and m == MT - 1 and t == "t3":
                        continue
                    ps[(m, t)] = ps_pool.tile([128, NSH], F32,
                                              name=f"ps_{m}_{t}")
            if NSPLIT != 0:
                # m1-t3 accumulates as two N-halves in separate banks
                # so the PE never writes a bank the DVE is reading
                # during the tail
                HN = NSH // 2
                ps_t3h = [
                    ps_pool.tile([128, NSH], F32, name=f"ps_t3h{h}")
                    for h in range(2)
                ]
            ps_warm = ps_pool.tile([128, NSH], F32, name="ps_warm")

            # PE warmup: one small scratch memset on the DVE queue
            # (idle until the first A_s prep ~2us later); N=128 warmup
            # matmuls ramp the HAM clock-gate while input DMA streams,
            # with fine granularity at the warmup -> real handoff.
            nc.vector.memset(warm_w[:], 0.0)
            for _ in range(NWARM):
                nc.tensor.matmul(ps_warm[:, 0:128], warm_w[:], warm_w[:],
                                 start=True, stop=True)

            # input DMA issue sequence: per-chunk B issues with the
            # coalesced A segment issues interleaved at A_POS
            k0 = 0
            b_slices = []
            for ch in CHUNKS:
                b_slices.append(slice(k0, k0 + ch))
                k0 += ch
            a_slices = [slice(lo, hi) for lo, hi in zip(A_SEG[:-1], A_SEG[1:])]
            n_chunks = len(CHUNKS)
            BH = min(BHALF_CHUNKS, n_chunks - 1)
            HN = NSH // 2
            issue_order = []  # ("A"|"B"|"B0"|"B1", slice)
            ai = 0
            for bi_, bsl in enumerate(b_slices):
                while ai < len(a_slices) and A_POS[ai] <= bi_:
                    issue_order.append(("A", a_slices[ai]))
                    ai += 1
                if bi_ < BH:
                    issue_order.append(("B0", bsl))
                    if bi_ == BH - 1:
                        for bsl2 in b_slices[:BH]:
                            issue_order.append(("B1", bsl2))
                else:
                    issue_order.append(("B", bsl))
            while ai < len(a_slices):
                issue_order.append(("A", a_slices[ai]))
                ai += 1
            for kind, sl in issue_order:
                if kind == "A":
                    nc.sync.dma_start(A_r[:, sl], a_r[:, sl])
                    nc.scalar.dma_start(A_i[:, sl], a_i[:, sl])
                elif kind == "B":
                    nc.sync.dma_start(B_r[:, sl], b_r[:, sl])
                    nc.scalar.dma_start(B_i[:, sl], b_i[:, sl])
                else:
                    h = 0 if kind == "B0" else 1
                    nsl = slice(h * HN, (h + 1) * HN)
                    nc.sync.dma_start(B_r[:, sl, nsl], b_r[:, sl, nsl])
                    nc.scalar.dma_start(B_i[:, sl, nsl], b_i[:, sl, nsl])

            operands = {
                "t1": (A_r, B_r),
                "t2": (A_i, B_i),
                "t3": (A_s, B_s),
            }

            # early chunks in two N-half phases: the h0 matmuls of each
            # k-tile need only half the B bytes, so the PE rides closer
            # behind the DMA completion line
            chunk_k0 = [s.start for s in b_slices]
            for h in range(2):
                nsl = slice(h * HN, (h + 1) * HN)
                for ci in range(BH):
                    ksl = b_slices[ci]
                    k0 = chunk_k0[ci]
                    ch = ksl.stop - ksl.start
                    if h == 0:
                        nc.vector.tensor_tensor(
                            A_s[:, ksl], A_r[:, ksl], A_i[:, ksl],
                            mybir.AluOpType.add)
                    nc.vector.tensor_tensor(
                        B_s[:, ksl, nsl], B_r[:, ksl, nsl],
                        B_i[:, ksl, nsl], mybir.AluOpType.add)
                    for t in ("t1", "t2", "t3"):
                        lhs, rhs = operands[t]
                        for kk in range(ch):
                            k = k0 + kk
                            for m in range(MT):
                                msl = slice(m * 128, (m + 1) * 128)
                                if NSPLIT != 0 and m == MT - 1 and t == "t3":
                                    nc.tensor.matmul(
                                        ps_t3h[h][:, 0:HN],
                                        lhs[:, k, msl], rhs[:, k, nsl],
                                        start=(k == 0), stop=False,
                                    )
                                else:
                                    # h1 into an h0-started bank must
                                    # NOT use start (it would clear the
                                    # whole bank's has_written bits);
                                    # untouched columns overwrite anyway
                                    nc.tensor.matmul(
                                        ps[(m, t)][:, nsl],
                                        lhs[:, k, msl], rhs[:, k, nsl],
                                        start=(h == 0 and k == 0),
                                        stop=False,
                                        skip_group_check=True,
                                    )

            k0 = chunk_k0[BH] if BH < n_chunks else KT
            for ci in range(BH, n_chunks):
                ch = CHUNKS[ci]
                ksl = slice(k0, k0 + ch)
                nc.vector.tensor_tensor(A_s[:, ksl], A_r[:, ksl],
                                        A_i[:, ksl], mybir.AluOpType.add)
                nc.vector.tensor_tensor(B_s[:, ksl], B_r[:, ksl],
                                        B_i[:, ksl], mybir.AluOpType.add)

                last_chunk = ci == n_chunks - 1
                if not last_chunk:
                    # product-major: t1 needs only A_r + B_r, so the PE
                    # starts before A_i/B_i/prep land
                    for t in ("t1", "t2", "t3"):
                        lhs, rhs = operands[t]
                        for kk in range(ch):
                            k = k0 + kk
                            for m in range(MT):
                                msl = slice(m * 128, (m + 1) * 128)
                                if NSPLIT != 0 and m == MT - 1 and t == "t3":
                                    for h in range(2):
                                        nsl = slice(h * HN, (h + 1) * HN)
                                        nc.tensor.matmul(
                                            ps_t3h[h][:, 0:HN],
                                            lhs[:, k, msl],
                                            rhs[:, k, nsl],
                                            start=(k == 0), stop=False,
                                        )
                                else:
                                    nc.tensor.matmul(
                                        ps[(m, t)][:], lhs[:, k, msl],
                                        rhs[:, k, :], start=(k == 0),
                                        stop=False,
                                        skip_group_check=True,
                                    )
                else:
                    # m-major: finish all of m0 first so its epilogue
                    # overlaps m1's matmuls; per product, issue the
                    # epilogue ops that only need completed banks.
                    for m in range(MT):
                        msl = slice(m * 128, (m + 1) * 128)
                        t1s = sb.tile([128, NSH], F32, name=f"t1s{m}")
                        w = sb.tile([128, NSH], F32, name=f"w{m}")
                        out_r = sb.tile([128, NSH], ODT, name=f"out_r{m}")
                        out_i = sb.tile([128, NSH], ODT, name=f"out_i{m}")
                        split = NSPLIT != 0 and m == MT - 1
                        for t in ("t1", "t2", "t3"):
                            lhs, rhs = operands[t]
                            if t == "t3" and split:
                                HN = NSH // 2
                                for h in range(2):
                                    nsl = slice(h * HN, (h + 1) * HN)
                                    dst = ps_t3h[h][:, 0:HN]
                                    for kk in range(ch):
                                        k = k0 + kk
                                        nc.tensor.matmul(
                                            dst,
                                            lhs[:, k, msl],
                                            rhs[:, k, nsl],
                                            start=(k == 0),
                                            stop=(kk == ch - 1),
                                        )
                                    nc.vector.tensor_tensor(
                                        out_i[:, nsl], dst,
                                        w[:, nsl],
                                        mybir.AluOpType.subtract)
                                    # route the two half-stores via
                                    # different queues so the last
                                    # issue isn't stuck behind the
                                    # first on one engine
                                    eng = nc.scalar if h == 0 else nc.sync
                                    eng.dma_start(
                                        c_i[msl, nsl], out_i[:, nsl])
                                continue
                            for kk in range(ch):
                                k = k0 + kk
                                nc.tensor.matmul(
                                    ps[(m, t)][:], lhs[:, k, msl],
                                    rhs[:, k, :], start=(k == 0),
                                    stop=(kk == ch - 1),
                                    skip_group_check=True,
                                )
                            if t == "t1":
                                nc.scalar.copy(t1s[:], ps[(m, "t1")][:])
                            elif t == "t2":
                                # C_r = t1 - t2; w = t1 + t2
                                nc.vector.tensor_tensor(
                                    out_r[:], t1s[:], ps[(m, "t2")][:],
                                    mybir.AluOpType.subtract)
                                nc.sync.dma_start(c_r[msl, :], out_r[:])
                                nc.vector.tensor_tensor(
                                    w[:], t1s[:], ps[(m, "t2")][:],
                                    mybir.AluOpType.add)
                            else:
                                # C_i = t3 - t1 - t2 = t3 - w
                                nc.vector.tensor_tensor(
                                    out_i[:], ps[(m, "t3")][:], w[:],
                                    mybir.AluOpType.subtract)
                                nc.scalar.dma_start(
                                    c_i[msl, :], out_i[:])
                k0 += ch

    nc.compile()
    return nc


def _get_nc(dt_name):
    if dt_name not in _CACHE:
        _CACHE[dt_name] = _build(dt_name)
    return _CACHE[dt_name]


def _pack_kxm(mat_t, np_dt):
    # mat_t: [4096, F] (k-major) -> [128, KT, F] with k = kt*128 + p
    f = mat_t.shape[1]
    return np.ascontiguousarray(
        mat_t.reshape(KT, 128, f).transpose(1, 0, 2).astype(np_dt)
    )


def run_device(A, B, dt_name=DT_NAME, trace=False):
    """A: [256, 4096] complex64, B: [4096, 4096] complex64.
    Returns C = A @ B as [256, 4096] complex64 plus the raw results."""
    from concourse import bass_utils

    nc = _get_nc(dt_name)
    np_dt = _np_dtype(dt_name)

    at = A.T  # [4096, 256]
    a_r = _pack_kxm(np.ascontiguousarray(at.real), np_dt)
    a_i = _pack_kxm(np.ascontiguousarray(at.imag), np_dt)
    br_full = B.real
    bi_full = B.imag

    in_maps = []
    for c in range(N_CORES):
        csl = slice(c * NSH, (c + 1) * NSH)
        in_maps.append({
            "a_r": a_r,
            "a_i": a_i,
            "b_r": _pack_kxm(np.ascontiguousarray(br_full[:, csl]), np_dt),
            "b_i": _pack_kxm(np.ascontiguousarray(bi_full[:, csl]), np_dt),
        })

    res = bass_utils.run_bass_kernel_spmd(
        nc, in_maps, core_ids=list(range(N_CORES)), trace=trace
    )

    out = np.empty((BATCH, HALF), dtype=np.complex64)
    for c in range(N_CORES):
        csl = slice(c * NSH, (c + 1) * NSH)
        out.real[:, csl] = res.results[c]["c_r"].astype(np.float32)
        out.imag[:, csl] = res.results[c]["c_i"].astype(np.float32)
    return out, res


def kernel(state, target_matrix, control, num_qubits):
    state = np.asarray(state)
    target_matrix = np.asarray(target_matrix)
    control = int(control)
    num_qubits = int(num_qubits)
    dim = 1 << num_qubits

    assert state.shape == (BATCH, DIM) and dim == DIM, (
        "kernel hardcoded for [256, 8192]"
    )

    mask = 1 << (num_qubits - control - 1)
    idx = np.arange(dim)
    c1 = idx[(idx & mask) != 0]  # columns with control bit set

    if control == 0:
        A = state[:, HALF:]
        B = target_matrix[HALF:, HALF:]
    else:
        A = state[:, c1]
        B = target_matrix[np.ix_(c1, c1)]
    A = np.ascontiguousarray(A, dtype=np.complex64)
    B = np.ascontiguousarray(B, dtype=np.complex64)

    C, _ = run_device(A, B)

    out = state.astype(np.complex64, copy=True)
    out[:, c1] = C
    return out
